# revision 27
# baseline (speedup 1.0000x reference)
"""Causal self-attention on 8 TRN2 NeuronCores — v2.

Reference computation (B=4, T=2048, C=1024, H=16, D=64, fp32):
    qkv = x @ W_attn + b_attn ; split q,k,v ; per-head causal softmax(q k^T / 8) @ v
    y = heads @ W_proj + b_proj

Sharding: core c handles batch b = c//2 and head-half hh = c%2 (8 heads).
QKV weights column-split, W_proj row-split; host sums the two partial
projections per batch and adds the constant bias (b_proj + b_v @ W_proj,
exact because softmax rows sum to 1). No collectives.

v2 design notes (cost-model driven; matmul cost = out_free x cycles/row,
fp8e4+DoubleRow = 0.5 cycles/row regardless of contraction size):
  - q,k generated with fp8 DoubleRow matmuls (x8/wq8/wk8 fp8, weights
    pre-scaled x32 so W~0.02 escapes fp8 subnormals; biases x32 to match;
    the 1/1024 descale is folded into the exp scale).
  - v generated in bf16 (value path dominates the error budget).
  - QK^T uses "padded DoubleRow": per head the contraction is d=64, so
    operands are [64 partitions, 2 k-subtiles, N] with subtile 1 an
    all-zero plane (host-DMA'd zeros) -> 0.5 cycles/row at K=64.
  - Causal mask is folded into PSUM before exp via a rank-127 constant
    bf16 matmul that adds -3e7 above the diagonal; exp then flushes those
    to 0. No DVE masking anywhere.
  - exp runs on chunk PAIRS ([128, 2048] ACT ops over a 4-bank PSUM tile)
    to amortize the ACT access-latency; per-chunk q-trimming via a 4D AP.
  - AV is computed in the SWAPPED layout: stationary = exp-scores chunk
    [kt, q], moving = v_aug [kt, 64 v | 1] -> out y[q, d|den] with free
    size 65 (half the cost of the yT layout), denominator per PARTITION.
    Normalize is then a cheap per-partition tensor_scalar; yT for the
    projection is rebuilt with identity matmul transposes (head B lands
    at partition offset 64 via tile_position).
  - PSUM→SBUF traffic and copies are spread across DVE and Pool (GPSIMD)
    to keep everything under the ACT exp floor (~131 us), the binding
    engine for this kernel.
"""

import numpy as np
import ml_dtypes

import concourse.bacc as bacc
import concourse.mybir as mybir
import concourse.tile as tile
from concourse.bass_utils import run_bass_kernel_spmd

F32 = mybir.dt.float32
BF16 = mybir.dt.bfloat16
FP8 = mybir.dt.float8e4
AF = mybir.ActivationFunctionType
ALU = mybir.AluOpType
DR = mybir.MatmulPerfMode.DoubleRow

N_CORES = 8
B, T, C = 4, 2048, 1024
H, D = 16, 64
CH = 512            # features per core (8 heads * 64)
NFO = 4             # head-pair chunks of 128 features
NTQ = 4             # t quarters of 512
NTC = 16            # t chunks of 128
WS = 32.0           # host-side weight prescale for fp8
EXP_SCALE = 0.125 / (WS * WS)   # softmax 1/8, undoing q,k x32 scales
MBIG = -3.0e7       # causal mask addend (pre-scale): exp -> 0

_cached = {}


def _build_nc():
    nc = bacc.Bacc("TRN2", debug=False, num_devices=N_CORES)

    d_xT = nc.dram_tensor("xT", [C, T], BF16, kind="ExternalInput")
    d_x8 = nc.dram_tensor("x8", [C, T], FP8, kind="ExternalInput")
    d_wq8 = nc.dram_tensor("wq8", [C, CH], FP8, kind="ExternalInput")
    d_wk8 = nc.dram_tensor("wk8", [C, CH], FP8, kind="ExternalInput")
    d_wv = nc.dram_tensor("wv", [C, CH], BF16, kind="ExternalInput")
    d_bq = nc.dram_tensor("bq", [128, NFO], F32, kind="ExternalInput")
    d_bk = nc.dram_tensor("bk", [128, NFO], F32, kind="ExternalInput")
    d_wp = nc.dram_tensor("wp", [CH, C], BF16, kind="ExternalInput")
    # consts [128, 896]: [tri | zeros | bmask(-3e7*I) | zeros x3 | iden]
    d_consts = nc.dram_tensor("consts", [128, 896], BF16, kind="ExternalInput")
    d_zero8 = nc.dram_tensor("zero8", [128, T], FP8, kind="ExternalInput")
    d_out = nc.dram_tensor("out", [T, C], F32, kind="ExternalOutput")

    with tile.TileContext(nc) as tc, nc.allow_low_precision(
        reason="fp8/bf16 staging; accumulation stays fp32 in PSUM"
    ), (
        tc.tile_pool(name="persist", bufs=1)
    ) as persist, (
        tc.tile_pool(name="pW", bufs=1)
    ) as pW, (
        tc.tile_pool(name="pX", bufs=1)
    ) as pX, (
        tc.tile_pool(name="pO", bufs=3)
    ) as pO, (
        tc.tile_pool(name="p2e", bufs=8)
    ) as p2e, (
        tc.tile_pool(name="p2r", bufs=2)
    ) as p2r, (
        tc.tile_pool(name="p2y", bufs=4)
    ) as p2y, (
        tc.tile_pool(name="psA", bufs=2, space="PSUM")
    ) as psA, (
        tc.tile_pool(name="psS", bufs=2, space="PSUM")
    ) as psS, (
        tc.tile_pool(name="psY", bufs=2, space="PSUM")
    ) as psY:
        # persistent on-chip tensors
        qT8 = [persist.tile([128, 2, T], FP8, tag=f"qT{fo}", name=f"qT{fo}") for fo in range(NFO)]
        kT8 = [persist.tile([128, 2, T], FP8, tag=f"kT{fo}", name=f"kT{fo}") for fo in range(NFO)]
        v = [persist.tile([128, 8, 65], BF16, tag=f"v{i}", name=f"v{i}") for i in range(NTC)]
        yT = persist.tile([128, NFO, T], BF16, tag="yT", name="yT")
        bq_sb = persist.tile([128, NFO], F32, tag="bq")
        bk_sb = persist.tile([128, NFO], F32, tag="bk")
        consts = persist.tile([128, 896], BF16, tag="consts")
        wq8_sb = pW.tile([128, 8, CH], FP8, tag="wq8")
        wk8_sb = pW.tile([128, 8, CH], FP8, tag="wk8")
        wv_sb = pW.tile([128, 8, CH], BF16, tag="wv")
        wp_sb = pW.tile([128, 4, C], BF16, tag="wp")
        x_tiles = [pX.tile([128, 8, 512], BF16, tag=f"x{tq}", name=f"x{tq}") for tq in range(NTQ)]
        x8_tiles = [pX.tile([128, 8, 512], FP8, tag=f"x8{tq}", name=f"x8{tq}") for tq in range(NTQ)]

        tri = consts[:, 0:128]          # tri[j, kt] = (kt > j)
        iden = consts[:, 768:896]
        # bmask rows: c0 diag chunk reads consts[256:256+N] = [-3e7*I | 0...],
        # c1 diag chunk reads consts[128:128+N] = [0 | -3e7*I | 0...]

        def _w_piece(dst, src, c0, c1):
            nc.sync.dma_start(
                dst[:, c0:c1, :],
                src.ap()[128 * c0 : 128 * c1, :].rearrange("(c p) f -> p c f", p=128),
            )

        # input DMAs: first-needed tensors split for a fast first matmul
        nc.sync.dma_start(
            x8_tiles[0][:, 0:4, :],
            d_x8.ap()[0:512, 0:512].rearrange("(c p) t -> p c t", p=128),
        )
        _w_piece(wq8_sb, d_wq8, 0, 4)
        nc.sync.dma_start(bq_sb[:], d_bq.ap())
        nc.sync.dma_start(bk_sb[:], d_bk.ap())
        nc.sync.dma_start(
            x8_tiles[0][:, 4:8, :],
            d_x8.ap()[512:1024, 0:512].rearrange("(c p) t -> p c t", p=128),
        )
        _w_piece(wq8_sb, d_wq8, 4, 8)
        nc.sync.dma_start(qT8[0][:, 1, :], d_zero8.ap())
        _w_piece(wk8_sb, d_wk8, 0, 8)
        nc.sync.dma_start(kT8[0][:, 1, :], d_zero8.ap())
        nc.sync.dma_start(consts[:], d_consts.ap())
        nc.sync.dma_start(
            x_tiles[0][:],
            d_xT.ap()[:, 0:512].rearrange("(c p) t -> p c t", p=128),
        )
        _w_piece(wv_sb, d_wv, 0, 8)
        for fo in range(1, NFO):
            nc.sync.dma_start(qT8[fo][:, 1, :], d_zero8.ap())
            nc.sync.dma_start(kT8[fo][:, 1, :], d_zero8.ap())
        for tq in range(1, NTQ):
            nc.sync.dma_start(
                x8_tiles[tq][:],
                d_x8.ap()[:, 512 * tq : 512 * (tq + 1)].rearrange("(c p) t -> p c t", p=128),
            )
            nc.sync.dma_start(
                x_tiles[tq][:],
                d_xT.ap()[:, 512 * tq : 512 * (tq + 1)].rearrange("(c p) t -> p c t", p=128),
            )
        nc.sync.dma_start(wp_sb[:], d_wp.ap().rearrange("(c p) f -> p c f", p=128))

        def emit_qk_gen(bq_, w_sb, b_sb, dstl, fo):
            # fp8 DoubleRow: 4 matmuls contract 2 ci-subtiles each
            ps = psA.tile([128, 512], F32, tag="psA", name="ps_qk")
            for j in range(4):
                nc.tensor.matmul(
                    ps[:],
                    w_sb[:, 2 * j : 2 * j + 2, 128 * fo : 128 * (fo + 1)],
                    x8_tiles[bq_][:, 2 * j : 2 * j + 2, :],
                    start=(j == 0),
                    stop=(j == 3),
                    perf_mode=DR,
                )
            nc.vector.tensor_scalar(
                dstl[fo][:, 0, 512 * bq_ : 512 * (bq_ + 1)],
                ps[:],
                b_sb[:, fo : fo + 1],
                None,
                op0=ALU.add,
            )

        def emit_v_group(bq_, ts_):
            tci = 4 * bq_ + ts_
            ps = psA.tile([128, 512], F32, tag="psA", name="ps_v")
            for ci in range(8):
                nc.tensor.matmul(
                    ps[:],
                    x_tiles[bq_][:, ci, 128 * ts_ : 128 * (ts_ + 1)],
                    wv_sb[:, ci, :],
                    start=(ci == 0),
                    stop=(ci == 7),
                )
            nc.vector.memset(v[tci][:, :, 64:65], 1.0)
            nc.vector.tensor_copy(
                v[tci][:, :, 0:64],
                ps[:].rearrange("p (h d) -> p h d", h=8),
            )

        def emit_qkv_group(bq_, g):
            if g < 4:
                emit_qk_gen(bq_, wq8_sb, bq_sb, qT8, g)
            elif g < 8:
                emit_qk_gen(bq_, wk8_sb, bk_sb, kT8, g - 4)
            else:
                emit_v_group(bq_, g - 8)

        def emit_proj_tc(tci):
            o_sb = pO.tile([128, C], F32, tag="o", name="o_sb")
            for co in range(2):
                ps = psA.tile([128, 512], F32, tag="psA", name="ps_o")
                for fo in range(NFO):
                    nc.tensor.matmul(
                        ps[:],
                        yT[:, fo, 128 * tci : 128 * (tci + 1)],
                        wp_sb[:, fo, 512 * co : 512 * (co + 1)],
                        start=(fo == 0),
                        stop=(fo == 3),
                    )
                nc.vector.tensor_copy(o_sb[:, 512 * co : 512 * (co + 1)], ps[:])
                nc.sync.dma_start(
                    d_out.ap()[128 * tci : 128 * (tci + 1), 512 * co : 512 * (co + 1)],
                    o_sb[:, 512 * co : 512 * (co + 1)],
                )

        def emit_attn_chunk(fo, b, c, pyh, av_defer=None):
            """QK + mask + exp + AV for k-chunk c (128 keys) of block b."""
            i = c - 4 * b                        # diag index (>=0: diagonal)
            qoff = 128 * max(i, 0)
            q0 = 512 * b
            pS = psS.tile([128, 1024], F32, tag="pS", name="pS")
            for h in range(2):
                nc.tensor.matmul(
                    pS[:, 512 * h + qoff : 512 * h + 512],
                    kT8[fo][64 * h : 64 * h + 64, :, 128 * c : 128 * (c + 1)],
                    qT8[fo][64 * h : 64 * h + 64, :, q0 + qoff : q0 + 512],
                    start=True,
                    stop=(i < 0),
                    perf_mode=DR,
                )
                if i >= 0:
                    # rank-127 mask matmul adds -3e7 above the diagonal of
                    # this chunk's own 128-wide q block
                    nc.tensor.matmul(
                        pS[:, 512 * h + qoff : 512 * h + qoff + 128],
                        tri,
                        consts[:, 256:384],
                        start=False,
                        stop=True,
                        skip_group_check=True,
                    )
            eST = p2e.tile([128, 1024], BF16, tag="eST", name="eST")
            pS3 = pS[:].rearrange("p (h q) -> p h q", h=2)
            eST3 = eST[:].rearrange("p (h q) -> p h q", h=2)
            nc.scalar.activation(
                eST3[:, :, qoff:512], pS3[:, :, qoff:512], AF.Exp, scale=EXP_SCALE
            )
            # PSUM groups are bank-granular: each head's [128,4,65] bank is ONE
            # accumulation group — start on the first write (qsub0 of chunk 0),
            # stop on the last (qsub3 of the final chunk); pending-zero handles
            # the disjoint qsub regions.
            def _avs():
                for qsub in range(max(i, 0), 4):
                    hb, l = divmod(qsub, 2)
                    for h in range(2):
                        # one accumulation group per bank: exactly one
                        # start (first write) and one stop (last write)
                        nc.tensor.matmul(
                            pyh[hb][:, l, h, :],
                            eST[:, 512 * h + 128 * qsub : 512 * h + 128 * (qsub + 1)],
                            v[c][:, 2 * fo + h, :],
                            start=(l == 0 and h == 0 and c == 0),
                            stop=(l == 1 and h == 1 and c == 4 * b + 2 * hb + 1),
                        )
            if av_defer is None:
                _avs()
            else:
                av_defer.append(_avs)

        def emit_half_tail1(fo, b, hb, py):
            """normalize one 256-query half: per-partition reciprocal of the
            denominator column, then y/den into SBUF."""
            ys = []
            for h in range(2):
                rec = p2r.tile([128, 2, 1], F32, tag="rec", name=f"rec{h}")
                nc.vector.reciprocal(rec[:], py[:, :, h, 64:65])
                y_sb = p2y.tile([128, 2, 64], BF16, tag="y_sb", name=f"y_sb{h}")
                for l in range(2):
                    nc.vector.tensor_scalar(
                        y_sb[:, l, :],
                        py[:, l, h, 0:64],
                        rec[:, l, :],
                        None,
                        op0=ALU.mult,
                    )
                ys.append(y_sb)
            return ys

        def emit_half_tail2(fo, b, hb, ys):
            """identity-matmul transposes of one half into yT (deferred so the
            PE wait-queue isn't clogged while the DVE normalizes)."""
            q0 = 512 * b + 256 * hb
            # one PSUM group per partition-half of the bank (h=0: parts 0:64,
            # h=1: parts 64:128); l regions share the group via pending-zero
            tp = psA.tile([128, 512], F32, tag="psA", name="tp")
            for l in range(2):
                for h in range(2):
                    nc.tensor.matmul(
                        tp[64 * h : 64 * h + 64, 128 * l : 128 * (l + 1)],
                        ys[h][:, l, :],
                        iden,
                        start=(l == 0),
                        stop=(l == 1),
                    )
            nc.vector.tensor_copy(yT[:, fo, q0 : q0 + 256], tp[:, 0:256])

        # Slot-based scheduling with lookahead: gen units are emitted LEAD
        # chunk-slots before their first consumer, so PE load stays level
        # (ACT, the exp floor, is the binding engine) while dependency chains
        # keep slack. Half-block AV accumulators (256 queries, one PSUM bank
        # for both heads, bufs=2) let each half's normalize/transpose tail
        # drain while the other half still accumulates.
        from collections import defaultdict as _dd
        slots_order = []
        for b_ in range(NTQ):
            for fo_ in range(NFO):
                for c_ in range(4 * b_ + 4):
                    slots_order.append((b_, fo_, c_))
        slot_of = {t: i for i, t in enumerate(slots_order)}
        LEAD = 3
        pre = _dd(list)
        skip = {(0, 0), (0, 4), (0, 1), (0, 5), (0, 8), (0, 9), (0, 10), (0, 11)}
        for b_ in range(NTQ):
            for fo_ in range(NFO):
                for g_, dl in ((fo_, slot_of[(b_, fo_, 0)]),
                               (4 + fo_, slot_of[(b_, fo_, 4 * b_)])):
                    if (b_, g_) in skip:
                        continue
                    pre[max(0, dl - LEAD)].append((dl, b_, g_))
            for ts_ in range(4):
                if (b_, 8 + ts_) in skip:
                    continue
                dl = slot_of[(b_, 0, 4 * b_ + ts_)]
                pre[max(0, dl - LEAD)].append((dl, b_, 8 + ts_))
        proj_slots = {1: [0, 1], 2: [2, 3, 4, 5], 3: [6, 7, 8, 9, 10, 11]}
        pending = []      # deferred transposes: (fo, b, hb, ys)

        def tail1_now(fo_, b_, hb_, py_):
            ys_ = emit_half_tail1(fo_, b_, hb_, py_)
            pending.append((fo_, b_, hb_, ys_))

        def pop_pending(n=1):
            for _ in range(min(n, len(pending))):
                emit_half_tail2(*pending.pop(0))

        # b=0 startup: fo0+fo1 QK/mask/exp first (they only need x8/w8 DMAs),
        # v-gen + the deferred AVs after — exp starts during the xT/wv DMA
        # wait instead of the PE wait-queue blocking on it.
        # psY tiles are allocated only right before their (deferred) AV
        # writes are emitted — pool ring generations follow emission order,
        # so allocating the next generation before the previous one's writes
        # would race. The av lists late-bind pyh via these mutable lists.
        av0 = []
        pyh_f0 = []
        emit_qkv_group(0, 0)
        emit_qkv_group(0, 4)
        for c in range(4):
            for dl, gb, gg in sorted(pre.get(slot_of[(0, 0, c)], [])):
                emit_qkv_group(gb, gg)
            emit_attn_chunk(0, 0, c, pyh_f0, av0)
        emit_qkv_group(0, 1)
        emit_qkv_group(0, 5)
        for ts in range(4):
            emit_qkv_group(0, 8 + ts)
        pyh_f0.extend(psY.tile([128, 2, 2, 65], F32, tag="pyh", name="pyh") for _ in range(2))
        for f in av0:
            f()
        tail1_now(0, 0, 0, pyh_f0[0])
        tail1_now(0, 0, 1, pyh_f0[1])

        for b in range(NTQ):
            nchunks = 4 * b + 4
            projq = list(proj_slots.get(b, []))
            for fo in range(NFO):
                if b == 0 and fo < 1:
                    continue     # handled in the startup block above
                for dl, gb, gg in sorted(pre.get(slot_of[(b, fo, 0)], [])):
                    emit_qkv_group(gb, gg)
                pyh = [psY.tile([128, 2, 2, 65], F32, tag="pyh", name="pyh") for _ in range(2)]
                for c in range(nchunks):
                    if c > 0:
                        for dl, gb, gg in sorted(pre.get(slot_of[(b, fo, c)], [])):
                            emit_qkv_group(gb, gg)
                    emit_attn_chunk(fo, b, c, pyh)
                    if pending and c in (1, 3):
                        pop_pending(1)
                    if c == 4 * b + 2 and b > 0:
                        # first half-block complete: normalize it now, its
                        # transposes join the deferred queue
                        tail1_now(fo, b, 0, pyh[0])
                    if projq and c % 4 == 2 and (b < 3 or (fo + c // 4) % 2 == 0):
                        t = projq.pop(0)
                        while any(p[1] == t // 4 for p in pending):
                            pop_pending(1)
                        emit_proj_tc(t)
                if b == 0:
                    tail1_now(fo, b, 0, pyh[0])
                tail1_now(fo, b, 1, pyh[1])
            while projq:
                t = projq.pop(0)
                while any(p[1] == t // 4 for p in pending):
                    pop_pending(1)
                emit_proj_tc(t)
        while pending:
            pop_pending(2)
        for tci in range(12, 16):
            emit_proj_tc(tci)

    nc.compile()
    return nc


def _get_nc():
    if "nc" not in _cached:
        _cached["nc"] = _build_nc()
    return _cached["nc"]


E4M3 = ml_dtypes.float8_e4m3fn


def _f8(a):
    return np.clip(np.ascontiguousarray(a, np.float32), -240, 240).astype(E4M3).view(np.uint8)


def _bf(a):
    return np.ascontiguousarray(a, np.float32).astype(ml_dtypes.bfloat16).view(np.uint16)


def kernel(x, W_attn, b_attn, W_proj, b_proj):
    x = np.asarray(x, np.float32)
    W_attn = np.asarray(W_attn, np.float32)
    b_attn = np.asarray(b_attn, np.float32)
    W_proj = np.asarray(W_proj, np.float32)
    b_proj = np.asarray(b_proj, np.float32)

    nc = _get_nc()
    j = np.arange(128)[:, None]
    kt = np.arange(128)[None, :]
    tri = (kt > j).astype(np.float32)            # [128,128]
    consts = np.zeros((128, 896), np.float32)
    consts[:, 0:128] = tri
    consts[:, 256:384] = MBIG * np.eye(128, dtype=np.float32)
    consts[:, 768:896] = np.eye(128, dtype=np.float32)
    consts_u16 = _bf(consts)
    zero8 = np.zeros((128, T), np.uint8)

    in_maps = []
    for c in range(N_CORES):
        b, hh = divmod(c, 2)
        sl = slice(CH * hh, CH * (hh + 1))
        xb = np.ascontiguousarray(x[b].T)
        in_maps.append(
            {
                "xT": _bf(xb),
                "x8": _f8(xb),
                "wq8": _f8(WS * W_attn[:, 0:C][:, sl]),
                "wk8": _f8(WS * W_attn[:, C : 2 * C][:, sl]),
                "wv": _bf(W_attn[:, 2 * C : 3 * C][:, sl]),
                "bq": np.ascontiguousarray(WS * b_attn[0:C][sl].reshape(NFO, 128).T),
                "bk": np.ascontiguousarray(WS * b_attn[C : 2 * C][sl].reshape(NFO, 128).T),
                "wp": _bf(W_proj[sl, :]),
                "consts": consts_u16,
                "zero8": zero8,
            }
        )

    try:
        res = run_bass_kernel_spmd(nc, in_maps, core_ids=list(range(N_CORES)))
    except Exception:
        # transient NRT device wedges happen; one retry is usually enough
        res = run_bass_kernel_spmd(nc, in_maps, core_ids=list(range(N_CORES)))

    bv = b_attn[2 * C : 3 * C]
    const_bias = (bv @ W_proj + b_proj).astype(np.float32)  # [C]
    out = np.empty((B, T, C), np.float32)
    for b in range(B):
        out[b] = res.results[2 * b]["out"] + res.results[2 * b + 1]["out"] + const_bias
    return out


# revision 28
# speedup vs baseline: 1.0112x; 1.0112x over previous
"""Causal self-attention on 8 TRN2 NeuronCores — v2.

Reference computation (B=4, T=2048, C=1024, H=16, D=64, fp32):
    qkv = x @ W_attn + b_attn ; split q,k,v ; per-head causal softmax(q k^T / 8) @ v
    y = heads @ W_proj + b_proj

Sharding: core c handles batch b = c//2 and head-half hh = c%2 (8 heads).
QKV weights column-split, W_proj row-split; host sums the two partial
projections per batch and adds the constant bias (b_proj + b_v @ W_proj,
exact because softmax rows sum to 1). No collectives.

v2 design notes (cost-model driven; matmul cost = out_free x cycles/row,
fp8e4+DoubleRow = 0.5 cycles/row regardless of contraction size):
  - q,k generated with fp8 DoubleRow matmuls (x8/wq8/wk8 fp8, weights
    pre-scaled x32 so W~0.02 escapes fp8 subnormals; biases x32 to match;
    the 1/1024 descale is folded into the exp scale).
  - v generated in bf16 (value path dominates the error budget).
  - QK^T uses "padded DoubleRow": per head the contraction is d=64, so
    operands are [64 partitions, 2 k-subtiles, N] with subtile 1 an
    all-zero plane (host-DMA'd zeros) -> 0.5 cycles/row at K=64.
  - Causal mask is folded into PSUM before exp via a rank-127 constant
    bf16 matmul that adds -3e7 above the diagonal; exp then flushes those
    to 0. No DVE masking anywhere.
  - exp runs on chunk PAIRS ([128, 2048] ACT ops over a 4-bank PSUM tile)
    to amortize the ACT access-latency; per-chunk q-trimming via a 4D AP.
  - AV is computed in the SWAPPED layout: stationary = exp-scores chunk
    [kt, q], moving = v_aug [kt, 64 v | 1] -> out y[q, d|den] with free
    size 65 (half the cost of the yT layout), denominator per PARTITION.
    Normalize is then a cheap per-partition tensor_scalar; yT for the
    projection is rebuilt with identity matmul transposes (head B lands
    at partition offset 64 via tile_position).
  - PSUM→SBUF traffic and copies are spread across DVE and Pool (GPSIMD)
    to keep everything under the ACT exp floor (~131 us), the binding
    engine for this kernel.
"""

import numpy as np
import ml_dtypes

import concourse.bacc as bacc
import concourse.mybir as mybir
import concourse.tile as tile
from concourse.bass_utils import run_bass_kernel_spmd

F32 = mybir.dt.float32
BF16 = mybir.dt.bfloat16
FP8 = mybir.dt.float8e4
AF = mybir.ActivationFunctionType
ALU = mybir.AluOpType
DR = mybir.MatmulPerfMode.DoubleRow

N_CORES = 8
B, T, C = 4, 2048, 1024
H, D = 16, 64
CH = 512            # features per core (8 heads * 64)
NFO = 4             # head-pair chunks of 128 features
NTQ = 4             # t quarters of 512
NTC = 16            # t chunks of 128
WS = 32.0           # host-side weight prescale for fp8
EXP_SCALE = 0.125 / (WS * WS)   # softmax 1/8, undoing q,k x32 scales
MBIG = -3.0e7       # causal mask addend (pre-scale): exp -> 0

_cached = {}


def _build_nc():
    nc = bacc.Bacc("TRN2", debug=False, num_devices=N_CORES)

    d_xT = nc.dram_tensor("xT", [C, T], BF16, kind="ExternalInput")
    d_x8 = nc.dram_tensor("x8", [C, T], FP8, kind="ExternalInput")
    d_wq8 = nc.dram_tensor("wq8", [C, CH], FP8, kind="ExternalInput")
    d_wk8 = nc.dram_tensor("wk8", [C, CH], FP8, kind="ExternalInput")
    d_wv = nc.dram_tensor("wv", [C, CH], BF16, kind="ExternalInput")
    d_bq = nc.dram_tensor("bq", [128, NFO], F32, kind="ExternalInput")
    d_bk = nc.dram_tensor("bk", [128, NFO], F32, kind="ExternalInput")
    d_wp = nc.dram_tensor("wp", [CH, C], BF16, kind="ExternalInput")
    # consts [128, 896]: [tri | zeros | bmask(-3e7*I) | zeros x3 | iden]
    d_consts = nc.dram_tensor("consts", [128, 896], BF16, kind="ExternalInput")
    d_zero8 = nc.dram_tensor("zero8", [128, T], FP8, kind="ExternalInput")
    d_out = nc.dram_tensor("out", [T, C], F32, kind="ExternalOutput")

    with tile.TileContext(nc) as tc, nc.allow_low_precision(
        reason="fp8/bf16 staging; accumulation stays fp32 in PSUM"
    ), (
        tc.tile_pool(name="persist", bufs=1)
    ) as persist, (
        tc.tile_pool(name="pW", bufs=1)
    ) as pW, (
        tc.tile_pool(name="pX", bufs=1)
    ) as pX, (
        tc.tile_pool(name="pO", bufs=3)
    ) as pO, (
        tc.tile_pool(name="p2e", bufs=8)
    ) as p2e, (
        tc.tile_pool(name="p2r", bufs=2)
    ) as p2r, (
        tc.tile_pool(name="p2y", bufs=4)
    ) as p2y, (
        tc.tile_pool(name="psA", bufs=2, space="PSUM")
    ) as psA, (
        tc.tile_pool(name="psS", bufs=2, space="PSUM")
    ) as psS, (
        tc.tile_pool(name="psY", bufs=2, space="PSUM")
    ) as psY:
        # persistent on-chip tensors
        qT8 = [persist.tile([128, 2, T], FP8, tag=f"qT{fo}", name=f"qT{fo}") for fo in range(NFO)]
        kT8 = [persist.tile([128, 2, T], FP8, tag=f"kT{fo}", name=f"kT{fo}") for fo in range(NFO)]
        v = [persist.tile([128, 8, 65], BF16, tag=f"v{i}", name=f"v{i}") for i in range(NTC)]
        yT = persist.tile([128, NFO, T], BF16, tag="yT", name="yT")
        bq_sb = persist.tile([128, NFO], F32, tag="bq")
        bk_sb = persist.tile([128, NFO], F32, tag="bk")
        consts = persist.tile([128, 896], BF16, tag="consts")
        wq8_sb = pW.tile([128, 8, CH], FP8, tag="wq8")
        wk8_sb = pW.tile([128, 8, CH], FP8, tag="wk8")
        wv_sb = pW.tile([128, 8, CH], BF16, tag="wv")
        wp_sb = pW.tile([128, 4, C], BF16, tag="wp")
        x_tiles = [pX.tile([128, 8, 512], BF16, tag=f"x{tq}", name=f"x{tq}") for tq in range(NTQ)]
        x8_tiles = [pX.tile([128, 8, 512], FP8, tag=f"x8{tq}", name=f"x8{tq}") for tq in range(NTQ)]

        tri = consts[:, 0:128]          # tri[j, kt] = (kt > j)
        iden = consts[:, 768:896]
        # bmask rows: c0 diag chunk reads consts[256:256+N] = [-3e7*I | 0...],
        # c1 diag chunk reads consts[128:128+N] = [0 | -3e7*I | 0...]

        def _w_piece(dst, src, c0, c1):
            nc.sync.dma_start(
                dst[:, c0:c1, :],
                src.ap()[128 * c0 : 128 * c1, :].rearrange("(c p) f -> p c f", p=128),
            )

        # input DMAs: first-needed tensors split for a fast first matmul
        nc.sync.dma_start(
            x8_tiles[0][:, 0:4, :],
            d_x8.ap()[0:512, 0:512].rearrange("(c p) t -> p c t", p=128),
        )
        _w_piece(wq8_sb, d_wq8, 0, 4)
        nc.sync.dma_start(bq_sb[:], d_bq.ap())
        nc.sync.dma_start(bk_sb[:], d_bk.ap())
        nc.sync.dma_start(
            x8_tiles[0][:, 4:8, :],
            d_x8.ap()[512:1024, 0:512].rearrange("(c p) t -> p c t", p=128),
        )
        _w_piece(wq8_sb, d_wq8, 4, 8)
        nc.sync.dma_start(qT8[0][:, 1, :], d_zero8.ap())
        _w_piece(wk8_sb, d_wk8, 0, 8)
        nc.sync.dma_start(kT8[0][:, 1, :], d_zero8.ap())
        nc.sync.dma_start(consts[:], d_consts.ap())
        nc.sync.dma_start(
            x_tiles[0][:],
            d_xT.ap()[:, 0:512].rearrange("(c p) t -> p c t", p=128),
        )
        _w_piece(wv_sb, d_wv, 0, 8)
        for fo in range(1, NFO):
            nc.sync.dma_start(qT8[fo][:, 1, :], d_zero8.ap())
            nc.sync.dma_start(kT8[fo][:, 1, :], d_zero8.ap())
        for tq in range(1, NTQ):
            nc.sync.dma_start(
                x8_tiles[tq][:],
                d_x8.ap()[:, 512 * tq : 512 * (tq + 1)].rearrange("(c p) t -> p c t", p=128),
            )
            nc.sync.dma_start(
                x_tiles[tq][:],
                d_xT.ap()[:, 512 * tq : 512 * (tq + 1)].rearrange("(c p) t -> p c t", p=128),
            )
        nc.sync.dma_start(wp_sb[:], d_wp.ap().rearrange("(c p) f -> p c f", p=128))

        def emit_qk_gen(bq_, w_sb, b_sb, dstl, fo):
            # fp8 DoubleRow: 4 matmuls contract 2 ci-subtiles each
            ps = psA.tile([128, 512], F32, tag="psA", name="ps_qk")
            for j in range(4):
                nc.tensor.matmul(
                    ps[:],
                    w_sb[:, 2 * j : 2 * j + 2, 128 * fo : 128 * (fo + 1)],
                    x8_tiles[bq_][:, 2 * j : 2 * j + 2, :],
                    start=(j == 0),
                    stop=(j == 3),
                    perf_mode=DR,
                )
            nc.vector.tensor_scalar(
                dstl[fo][:, 0, 512 * bq_ : 512 * (bq_ + 1)],
                ps[:],
                b_sb[:, fo : fo + 1],
                None,
                op0=ALU.add,
            )

        def emit_v_group(bq_, ts_):
            tci = 4 * bq_ + ts_
            ps = psA.tile([128, 512], F32, tag="psA", name="ps_v")
            for ci in range(8):
                nc.tensor.matmul(
                    ps[:],
                    x_tiles[bq_][:, ci, 128 * ts_ : 128 * (ts_ + 1)],
                    wv_sb[:, ci, :],
                    start=(ci == 0),
                    stop=(ci == 7),
                )
            nc.vector.memset(v[tci][:, :, 64:65], 1.0)
            nc.vector.tensor_copy(
                v[tci][:, :, 0:64],
                ps[:].rearrange("p (h d) -> p h d", h=8),
            )

        def emit_qkv_group(bq_, g):
            if g < 4:
                emit_qk_gen(bq_, wq8_sb, bq_sb, qT8, g)
            elif g < 8:
                emit_qk_gen(bq_, wk8_sb, bk_sb, kT8, g - 4)
            else:
                emit_v_group(bq_, g - 8)

        def emit_proj_tc(tci):
            o_sb = pO.tile([128, C], F32, tag="o", name="o_sb")
            for co in range(2):
                ps = psA.tile([128, 512], F32, tag="psA", name="ps_o")
                for fo in range(NFO):
                    nc.tensor.matmul(
                        ps[:],
                        yT[:, fo, 128 * tci : 128 * (tci + 1)],
                        wp_sb[:, fo, 512 * co : 512 * (co + 1)],
                        start=(fo == 0),
                        stop=(fo == 3),
                    )
                nc.vector.tensor_copy(o_sb[:, 512 * co : 512 * (co + 1)], ps[:])
                nc.sync.dma_start(
                    d_out.ap()[128 * tci : 128 * (tci + 1), 512 * co : 512 * (co + 1)],
                    o_sb[:, 512 * co : 512 * (co + 1)],
                )

        def emit_attn_chunk(fo, b, c, pyh, av_defer=None):
            """QK + mask + exp + AV for k-chunk c (128 keys) of block b."""
            i = c - 4 * b                        # diag index (>=0: diagonal)
            qoff = 128 * max(i, 0)
            q0 = 512 * b
            pS = psS.tile([128, 1024], F32, tag="pS", name="pS")
            for h in range(2):
                nc.tensor.matmul(
                    pS[:, 512 * h + qoff : 512 * h + 512],
                    kT8[fo][64 * h : 64 * h + 64, :, 128 * c : 128 * (c + 1)],
                    qT8[fo][64 * h : 64 * h + 64, :, q0 + qoff : q0 + 512],
                    start=True,
                    stop=(i < 0),
                    perf_mode=DR,
                )
                if i >= 0:
                    # rank-127 mask matmul adds -3e7 above the diagonal of
                    # this chunk's own 128-wide q block
                    nc.tensor.matmul(
                        pS[:, 512 * h + qoff : 512 * h + qoff + 128],
                        tri,
                        consts[:, 256:384],
                        start=False,
                        stop=True,
                        skip_group_check=True,
                    )
            eST = p2e.tile([128, 1024], BF16, tag="eST", name="eST")
            pS3 = pS[:].rearrange("p (h q) -> p h q", h=2)
            eST3 = eST[:].rearrange("p (h q) -> p h q", h=2)
            nc.scalar.activation(
                eST3[:, :, qoff:512], pS3[:, :, qoff:512], AF.Exp, scale=EXP_SCALE
            )
            # PSUM groups are bank-granular: each head's [128,4,65] bank is ONE
            # accumulation group — start on the first write (qsub0 of chunk 0),
            # stop on the last (qsub3 of the final chunk); pending-zero handles
            # the disjoint qsub regions.
            def _avs():
                for qsub in range(max(i, 0), 4):
                    hb, l = divmod(qsub, 2)
                    for h in range(2):
                        # one accumulation group per bank: exactly one
                        # start (first write) and one stop (last write)
                        nc.tensor.matmul(
                            pyh[hb][:, l, h, :],
                            eST[:, 512 * h + 128 * qsub : 512 * h + 128 * (qsub + 1)],
                            v[c][:, 2 * fo + h, :],
                            start=(l == 0 and h == 0 and c == 0),
                            stop=(l == 1 and h == 1 and c == 4 * b + 2 * hb + 1),
                        )
            if av_defer is None:
                _avs()
            else:
                av_defer.append(_avs)

        def emit_half_tail1(fo, b, hb, py):
            """normalize one 256-query half: per-partition reciprocal of the
            denominator column, then y/den into SBUF."""
            ys = []
            for h in range(2):
                rec = p2r.tile([128, 2, 1], F32, tag="rec", name=f"rec{h}")
                nc.vector.reciprocal(rec[:], py[:, :, h, 64:65])
                y_sb = p2y.tile([128, 2, 64], BF16, tag="y_sb", name=f"y_sb{h}")
                for l in range(2):
                    nc.vector.tensor_scalar(
                        y_sb[:, l, :],
                        py[:, l, h, 0:64],
                        rec[:, l, :],
                        None,
                        op0=ALU.mult,
                    )
                ys.append(y_sb)
            return ys

        def emit_half_tail2(fo, b, hb, ys):
            """identity-matmul transposes of one half into yT (deferred so the
            PE wait-queue isn't clogged while the DVE normalizes)."""
            q0 = 512 * b + 256 * hb
            # one PSUM group per partition-half of the bank (h=0: parts 0:64,
            # h=1: parts 64:128); l regions share the group via pending-zero
            tp = psA.tile([128, 512], F32, tag="psA", name="tp")
            for l in range(2):
                for h in range(2):
                    nc.tensor.matmul(
                        tp[64 * h : 64 * h + 64, 128 * l : 128 * (l + 1)],
                        ys[h][:, l, :],
                        iden,
                        start=(l == 0),
                        stop=(l == 1),
                    )
            nc.vector.tensor_copy(yT[:, fo, q0 : q0 + 256], tp[:, 0:256])

        # Slot-based scheduling with lookahead: gen units are emitted LEAD
        # chunk-slots before their first consumer, so PE load stays level
        # (ACT, the exp floor, is the binding engine) while dependency chains
        # keep slack. Half-block AV accumulators (256 queries, one PSUM bank
        # for both heads, bufs=2) let each half's normalize/transpose tail
        # drain while the other half still accumulates.
        from collections import defaultdict as _dd
        slots_order = []
        for b_ in range(NTQ):
            for fo_ in range(NFO):
                for c_ in range(4 * b_ + 4):
                    slots_order.append((b_, fo_, c_))
        slot_of = {t: i for i, t in enumerate(slots_order)}
        LEAD = 3
        pre = _dd(list)
        skip = {(0, 0), (0, 4), (0, 1), (0, 5), (0, 8), (0, 9), (0, 10), (0, 11)}
        for b_ in range(NTQ):
            for fo_ in range(NFO):
                for g_, dl in ((fo_, slot_of[(b_, fo_, 0)]),
                               (4 + fo_, slot_of[(b_, fo_, 4 * b_)])):
                    if (b_, g_) in skip:
                        continue
                    pre[max(0, dl - LEAD)].append((dl, b_, g_))
            for ts_ in range(4):
                if (b_, 8 + ts_) in skip:
                    continue
                dl = slot_of[(b_, 0, 4 * b_ + ts_)]
                pre[max(0, dl - LEAD)].append((dl, b_, 8 + ts_))
        proj_slots = {1: [0, 1], 2: [2, 3, 4, 5], 3: [6, 7, 8, 9, 10, 11]}
        pending = []      # deferred transposes: (fo, b, hb, ys)

        def tail1_now(fo_, b_, hb_, py_):
            ys_ = emit_half_tail1(fo_, b_, hb_, py_)
            pending.append((fo_, b_, hb_, ys_))

        def pop_pending(n=1):
            for _ in range(min(n, len(pending))):
                emit_half_tail2(*pending.pop(0))

        # b=0 startup: fo0+fo1 QK/mask/exp first (they only need x8/w8 DMAs),
        # v-gen + the deferred AVs after — exp starts during the xT/wv DMA
        # wait instead of the PE wait-queue blocking on it.
        # psY tiles are allocated only right before their (deferred) AV
        # writes are emitted — pool ring generations follow emission order,
        # so allocating the next generation before the previous one's writes
        # would race. The av lists late-bind pyh via these mutable lists.
        av0, av1 = [], []
        pyh_f0, pyh_f1 = [], []
        emit_qkv_group(0, 0)
        emit_qkv_group(0, 4)
        for c in range(4):
            for dl, gb, gg in sorted(pre.get(slot_of[(0, 0, c)], [])):
                emit_qkv_group(gb, gg)
            emit_attn_chunk(0, 0, c, pyh_f0, av0)
        emit_qkv_group(0, 1)
        emit_qkv_group(0, 5)
        for c in range(4):
            for dl, gb, gg in sorted(pre.get(slot_of[(0, 1, c)], [])):
                emit_qkv_group(gb, gg)
            emit_attn_chunk(0, 1, c, pyh_f1, av1)
        for ts in range(4):
            emit_qkv_group(0, 8 + ts)
        pyh_f0.extend(psY.tile([128, 2, 2, 65], F32, tag="pyh", name="pyh") for _ in range(2))
        for f in av0:
            f()
        tail1_now(0, 0, 0, pyh_f0[0])
        tail1_now(0, 0, 1, pyh_f0[1])
        pyh_f1.extend(psY.tile([128, 2, 2, 65], F32, tag="pyh", name="pyh") for _ in range(2))
        for f in av1:
            f()
        tail1_now(1, 0, 0, pyh_f1[0])
        tail1_now(1, 0, 1, pyh_f1[1])

        for b in range(NTQ):
            nchunks = 4 * b + 4
            projq = list(proj_slots.get(b, []))
            for fo in range(NFO):
                if b == 0 and fo < 2:
                    continue     # handled in the startup block above
                for dl, gb, gg in sorted(pre.get(slot_of[(b, fo, 0)], [])):
                    emit_qkv_group(gb, gg)
                pyh = [psY.tile([128, 2, 2, 65], F32, tag="pyh", name="pyh") for _ in range(2)]
                for c in range(nchunks):
                    if c > 0:
                        for dl, gb, gg in sorted(pre.get(slot_of[(b, fo, c)], [])):
                            emit_qkv_group(gb, gg)
                    emit_attn_chunk(fo, b, c, pyh)
                    if pending and c in (1, 3):
                        pop_pending(1)
                    if c == 4 * b + 2 and b > 0:
                        # first half-block complete: normalize it now, its
                        # transposes join the deferred queue
                        tail1_now(fo, b, 0, pyh[0])
                    if projq and c % 4 == 2 and (b < 3 or (fo + c // 4) % 2 == 0):
                        t = projq.pop(0)
                        while any(p[1] == t // 4 for p in pending):
                            pop_pending(1)
                        emit_proj_tc(t)
                if b == 0:
                    tail1_now(fo, b, 0, pyh[0])
                tail1_now(fo, b, 1, pyh[1])
            while projq:
                t = projq.pop(0)
                while any(p[1] == t // 4 for p in pending):
                    pop_pending(1)
                emit_proj_tc(t)
        while pending:
            pop_pending(2)
        for tci in range(12, 16):
            emit_proj_tc(tci)

    nc.compile()
    return nc


def _get_nc():
    if "nc" not in _cached:
        _cached["nc"] = _build_nc()
    return _cached["nc"]


E4M3 = ml_dtypes.float8_e4m3fn


def _f8(a):
    return np.clip(np.ascontiguousarray(a, np.float32), -240, 240).astype(E4M3).view(np.uint8)


def _bf(a):
    return np.ascontiguousarray(a, np.float32).astype(ml_dtypes.bfloat16).view(np.uint16)


def kernel(x, W_attn, b_attn, W_proj, b_proj):
    x = np.asarray(x, np.float32)
    W_attn = np.asarray(W_attn, np.float32)
    b_attn = np.asarray(b_attn, np.float32)
    W_proj = np.asarray(W_proj, np.float32)
    b_proj = np.asarray(b_proj, np.float32)

    nc = _get_nc()
    j = np.arange(128)[:, None]
    kt = np.arange(128)[None, :]
    tri = (kt > j).astype(np.float32)            # [128,128]
    consts = np.zeros((128, 896), np.float32)
    consts[:, 0:128] = tri
    consts[:, 256:384] = MBIG * np.eye(128, dtype=np.float32)
    consts[:, 768:896] = np.eye(128, dtype=np.float32)
    consts_u16 = _bf(consts)
    zero8 = np.zeros((128, T), np.uint8)

    in_maps = []
    for c in range(N_CORES):
        b, hh = divmod(c, 2)
        sl = slice(CH * hh, CH * (hh + 1))
        xb = np.ascontiguousarray(x[b].T)
        in_maps.append(
            {
                "xT": _bf(xb),
                "x8": _f8(xb),
                "wq8": _f8(WS * W_attn[:, 0:C][:, sl]),
                "wk8": _f8(WS * W_attn[:, C : 2 * C][:, sl]),
                "wv": _bf(W_attn[:, 2 * C : 3 * C][:, sl]),
                "bq": np.ascontiguousarray(WS * b_attn[0:C][sl].reshape(NFO, 128).T),
                "bk": np.ascontiguousarray(WS * b_attn[C : 2 * C][sl].reshape(NFO, 128).T),
                "wp": _bf(W_proj[sl, :]),
                "consts": consts_u16,
                "zero8": zero8,
            }
        )

    try:
        res = run_bass_kernel_spmd(nc, in_maps, core_ids=list(range(N_CORES)))
    except Exception:
        # transient NRT device wedges happen; one retry is usually enough
        res = run_bass_kernel_spmd(nc, in_maps, core_ids=list(range(N_CORES)))

    bv = b_attn[2 * C : 3 * C]
    const_bias = (bv @ W_proj + b_proj).astype(np.float32)  # [C]
    out = np.empty((B, T, C), np.float32)
    for b in range(B):
        out[b] = res.results[2 * b]["out"] + res.results[2 * b + 1]["out"] + const_bias
    return out
